# revision 22
# baseline (speedup 1.0000x reference)
"""Trainium2 Bass kernel for nn_DeltaVisionMambaBlock.

Self-contained: takes FULL unsharded inputs, returns FULL output.

Decomposition across 8 NeuronCores, two SPMD launches, no collectives:
  Launch 1 (token-sharded: core = batch b x token-quarter q):
    diff-prologue proj (Wp) -> RMSNorm -> in_proj (Win) -> depthwise causal
    conv -> silu -> x_proj (Wx) -> dt_proj (Wdt) -> softplus.
    Channel-transposed layout throughout ([ch, tok]) so no PE transposes.
  Host: reshard [ch, tok] quarters into per-batch [DI, N]; du = delta*u;
    replicate B/C rows across partitions.
  Launch 2 (d_inner-sharded: core = batch b x d-quarter j):
    selective scan via DVE tensor_tensor_scan (time chunks of 512, state
    chained via last-column handoff), y = sum_s (h_s * C_s) accumulated in
    PSUM via identity matmuls, gating, out_proj (Wout) partials.
  Host: sum the 4 d-shard partials per batch, add skip connection.
"""
import sys

if "/opt/trn_rl_repo" not in sys.path:
    sys.path.insert(0, "/opt/trn_rl_repo")

import numpy as np
import ml_dtypes

import concourse.bass as bass
import concourse.tile as tile
from concourse import bacc, mybir
from concourse.bass_utils import run_bass_kernel_spmd
from concourse.masks import make_identity

F32 = mybir.dt.float32
BF16 = mybir.dt.bfloat16
AF = mybir.ActivationFunctionType
OP = mybir.AluOpType

B, N, D = 2, 2048, 768
DI, DS, DC, DTR = 1536, 16, 4, 48
EPS = 1e-5
NCORES = 8
TQ = N // 4            # 512 tokens per launch-1 core
DSH = DI // 4          # 384 d-inner channels per launch-2 core
KP = 13 * 128          # padded contraction dim (2D + bias row -> 1664)
P = 128
TCH = 512              # launch-2 time chunk
NTCH = N // TCH


# ----------------------------------------------------------------------
# Launch 1 program
# ----------------------------------------------------------------------
def _build_launch1():
    nc = bacc.Bacc("TRN2", target_bir_lowering=False, debug=False,
                   num_devices=NCORES)
    combT = nc.dram_tensor("combT", [KP, TQ], BF16, kind="ExternalInput").ap()
    wp = nc.dram_tensor("wp", [KP, D], BF16, kind="ExternalInput").ap()
    win = nc.dram_tensor("win", [D, 2 * DI], BF16, kind="ExternalInput").ap()
    wcb = nc.dram_tensor("wcb", [DI, DC + 1 + DC - 1], F32,
                         kind="ExternalInput").ap()
    wx = nc.dram_tensor("wx", [DI, DTR + 2 * DS], BF16, kind="ExternalInput").ap()
    wdt = nc.dram_tensor("wdt", [DTR, DI], BF16, kind="ExternalInput").ap()

    u_out = nc.dram_tensor("u_out", [DI, TQ], BF16, kind="ExternalOutput").ap()
    delta_out = nc.dram_tensor("delta_out", [DI, TQ], F32, kind="ExternalOutput").ap()
    z_out = nc.dram_tensor("z_out", [DI, TQ], BF16, kind="ExternalOutput").ap()
    bc_out = nc.dram_tensor("bc_out", [2 * DS, TQ], BF16, kind="ExternalOutput").ap()

    NK = KP // P          # 13
    ND = D // P           # 6
    NDI = DI // P         # 12

    with tile.TileContext(nc) as tc:
        with (
            tc.tile_pool(name="const", bufs=1) as cpool,
            tc.tile_pool(name="work", bufs=2) as work,
            tc.tile_pool(name="persist", bufs=1) as persist,
            tc.tile_pool(name="ps", bufs=4, space="PSUM") as pspool,
            tc.tile_pool(name="ps1", bufs=1, space="PSUM") as ps1pool,
        ):
            ones_k = cpool.tile([P, 1], F32, tag="ones_k")
            nc.vector.memset(ones_k[:], 1.0)
            ones_r = cpool.tile([1, P], F32, tag="ones_r")
            nc.vector.memset(ones_r[:], 1.0)

            normed_sb = []
            # ---- stage 1: projT = (Wp_aug.T)^T @ combT, m=channels ----
            with tc.tile_pool(name="s1", bufs=1) as s1:
                combT_sb = []
                wp_sb = []
                for k in range(NK):
                    t = s1.tile([P, TQ], BF16, tag=f"combT{k}")
                    nc.sync.dma_start(t[:], combT[k * P:(k + 1) * P, :])
                    combT_sb.append(t)
                    w = s1.tile([P, D], BF16, tag=f"wp{k}")
                    nc.scalar.dma_start(w[:], wp[k * P:(k + 1) * P, :])
                    wp_sb.append(w)

                proj_sb = []
                sq_sb = []
                for m in range(ND):
                    ps = pspool.tile([P, TQ], F32, tag="mm_ps")
                    for k in range(NK):
                        nc.tensor.matmul(ps[:], wp_sb[k][:, m * P:(m + 1) * P],
                                         combT_sb[k][:], start=(k == 0),
                                         stop=(k == NK - 1))
                    p_sb = s1.tile([P, TQ], F32, tag=f"proj{m}")
                    nc.vector.tensor_copy(p_sb[:], ps[:])
                    proj_sb.append(p_sb)
                    s_sb = s1.tile([P, TQ], F32, tag=f"sq{m}")
                    nc.scalar.activation(s_sb[:], ps[:], AF.Square)
                    sq_sb.append(s_sb)

                ssq_ps = ps1pool.tile([1, TQ], F32, tag="ssq")
                for m in range(ND):
                    nc.tensor.matmul(ssq_ps[:], ones_k[:], sq_sb[m][:],
                                     start=(m == 0), stop=(m == ND - 1))
                eps_t = cpool.tile([1, 1], F32, tag="eps")
                nc.vector.memset(eps_t[:], EPS)
                sqrt_sb = work.tile([1, TQ], F32, tag="sqrt")
                nc.scalar.activation(sqrt_sb[:], ssq_ps[:], AF.Sqrt,
                                     bias=eps_t[:, 0:1], scale=1.0 / D)
                rms_sb = work.tile([1, TQ], F32, tag="rms")
                nc.vector.reciprocal(rms_sb[:], sqrt_sb[:])
                rms_ps = ps1pool.tile([P, TQ], F32, tag="rmsb")
                nc.tensor.matmul(rms_ps[:], ones_r[:], rms_sb[:],
                                 start=True, stop=True)

                for m in range(ND):
                    nt = persist.tile([P, TQ], BF16, tag=f"normed{m}")
                    nc.vector.tensor_mul(nt[:], proj_sb[m][:], rms_ps[:])
                    normed_sb.append(nt)

            # ---- stage 2: xzT = (Win_eff.T)^T @ normedT ----
            xi_ext = []
            wcb_t = []
            for mi in range(NDI):
                wc = cpool.tile([P, DC + 1], F32, tag=f"wcb{mi}")
                nc.sync.dma_start(wc[:], wcb[mi * P:(mi + 1) * P, 0:DC + 1])
                wcb_t.append(wc)
                xe = persist.tile([P, (DC - 1) + TQ], F32, tag=f"xiext{mi}")
                nc.sync.dma_start(xe[:, 0:DC - 1],
                                  wcb[mi * P:(mi + 1) * P, DC + 1:])
                xi_ext.append(xe)

            with tc.tile_pool(name="s2", bufs=1) as s2:
                win_sb = []
                for k in range(ND):
                    w = s2.tile([P, 2 * DI], BF16, tag=f"win{k}")
                    nc.scalar.dma_start(w[:], win[k * P:(k + 1) * P, :])
                    win_sb.append(w)

                for mi in range(2 * NDI):
                    ps = pspool.tile([P, TQ], F32, tag="mm_ps")
                    for k in range(ND):
                        nc.tensor.matmul(ps[:],
                                         win_sb[k][:, mi * P:(mi + 1) * P],
                                         normed_sb[k][:], start=(k == 0),
                                         stop=(k == ND - 1))
                    if mi < NDI:
                        nc.scalar.activation(xi_ext[mi][:, DC - 1:], ps[:],
                                             AF.Copy)
                    else:
                        z_sb = work.tile([P, TQ], BF16, tag="z")
                        nc.scalar.activation(z_sb[:], ps[:], AF.Copy)
                        j = mi - NDI
                        nc.gpsimd.dma_start(z_out[j * P:(j + 1) * P, :], z_sb[:])

            # ---- conv + silu -> u ----
            u_sb = []
            for mi in range(NDI):
                acc = work.tile([P, TQ], F32, tag="cacc")
                nc.vector.tensor_scalar(acc[:], xi_ext[mi][:, 0:TQ],
                                        wcb_t[mi][:, 0:1], None, OP.mult)
                for j in range(1, DC):
                    acc2 = work.tile([P, TQ], F32, tag="cacc")
                    nc.vector.scalar_tensor_tensor(
                        acc2[:], xi_ext[mi][:, j:j + TQ], wcb_t[mi][:, j:j + 1],
                        acc[:], OP.mult, OP.add)
                    acc = acc2
                ut = persist.tile([P, TQ], BF16, tag=f"u{mi}")
                nc.scalar.activation(ut[:], acc[:], AF.Silu,
                                     bias=wcb_t[mi][:, DC:DC + 1])
                u_sb.append(ut)
                nc.gpsimd.dma_start(u_out[mi * P:(mi + 1) * P, :], ut[:])

            # ---- x_dbl = (Wx.T)^T @ uT ----
            wx_sb = []
            for k in range(NDI):
                w = cpool.tile([P, DTR + 2 * DS], BF16, tag=f"wx{k}")
                nc.sync.dma_start(w[:], wx[k * P:(k + 1) * P, :])
                wx_sb.append(w)
            xdbl_ps = ps1pool.tile([DTR + 2 * DS, TQ], F32, tag="xdbl")
            for k in range(NDI):
                nc.tensor.matmul(xdbl_ps[:], wx_sb[k][:], u_sb[k][:],
                                 start=(k == 0), stop=(k == NDI - 1))
            xdbl_sb = persist.tile([DTR + 2 * DS, TQ], BF16, tag="xdbl_sb")
            nc.scalar.activation(xdbl_sb[:], xdbl_ps[:], AF.Copy)
            nc.sync.dma_start(bc_out[:], xdbl_sb[DTR:, :])

            # ---- dt_lin = Wdt @ dt (softplus+bias on host) ----
            wdt_sb = cpool.tile([DTR, DI], BF16, tag="wdt")
            nc.sync.dma_start(wdt_sb[:], wdt[:])
            for mi in range(NDI):
                ps = pspool.tile([P, TQ], F32, tag="mm_ps")
                nc.tensor.matmul(ps[:], wdt_sb[:, mi * P:(mi + 1) * P],
                                 xdbl_sb[0:DTR, :], start=True, stop=True)
                d_sb = work.tile([P, TQ], F32, tag="delta")
                nc.scalar.activation(d_sb[:], ps[:], AF.Copy)
                nc.gpsimd.dma_start(delta_out[mi * P:(mi + 1) * P, :], d_sb[:])

    nc.compile()
    return nc


# ----------------------------------------------------------------------
# Launch 2 program
# ----------------------------------------------------------------------
def _build_launch2():
    nc = bacc.Bacc("TRN2", target_bir_lowering=False, debug=False,
                   num_devices=NCORES)
    delta = nc.dram_tensor("delta", [DSH, N], F32, kind="ExternalInput").ap()
    ud = nc.dram_tensor("ud", [DSH, N], F32, kind="ExternalInput").ap()
    z = nc.dram_tensor("z", [DSH, N], BF16, kind="ExternalInput").ap()
    dbus = nc.dram_tensor("dbus", [DS, DSH, N], BF16, kind="ExternalInput").ap()
    crep = nc.dram_tensor("crep", [DS, P, N], BF16, kind="ExternalInput").ap()
    acol = nc.dram_tensor("acol", [DSH, DS], F32, kind="ExternalInput").ap()
    wout = nc.dram_tensor("wout", [DSH, D], BF16, kind="ExternalInput").ap()
    outp = nc.dram_tensor("outp", [D, N], F32, kind="ExternalOutput").ap()

    NCT = DSH // P        # 3

    with tile.TileContext(nc) as tc:
        with (
            tc.tile_pool(name="const", bufs=1) as cpool,
            tc.tile_pool(name="inp", bufs=1) as inp,
            tc.tile_pool(name="bc", bufs=2) as bcpool,
            tc.tile_pool(name="work", bufs=2) as work,
            tc.tile_pool(name="yz", bufs=1) as yzpool,
            tc.tile_pool(name="ps", bufs=2, space="PSUM") as pspool,
        ):
            ident = cpool.tile([P, P], BF16, tag="ident")
            make_identity(nc, ident[:])

            delta_t, ud_t, z_t, acol_t, wout_t = [], [], [], [], []
            for ct in range(NCT):
                r = slice(ct * P, (ct + 1) * P)
                t = inp.tile([P, N], F32, tag=f"dlt{ct}")
                nc.sync.dma_start(t[:], delta[r, :])
                delta_t.append(t)
                t = inp.tile([P, N], F32, tag=f"ud{ct}")
                nc.sync.dma_start(t[:], ud[r, :])
                ud_t.append(t)
                t = inp.tile([P, N], BF16, tag=f"z{ct}")
                nc.sync.dma_start(t[:], z[r, :])
                z_t.append(t)
                t = inp.tile([P, DS], F32, tag=f"acol{ct}")
                nc.sync.dma_start(t[:], acol[r, :])
                acol_t.append(t)
                t = inp.tile([P, D], BF16, tag=f"wout{ct}")
                nc.sync.dma_start(t[:], wout[r, :])
                wout_t.append(t)
            yz_t = [yzpool.tile([P, N], BF16, tag=f"yz{ct}", name=f"yz{ct}")
                    for ct in range(NCT)]
            hstate = [yzpool.tile([P, 1], F32, tag=f"hs{i}", name=f"hs{i}")
                      for i in range(NCT * DS)]

            H = N // 2
            for th in range(2):
                hs = slice(th * H, (th + 1) * H)
                y_ps = [pspool.tile([P, H], F32, tag=f"y{ct}", name=f"yps{ct}",
                                    bufs=1)
                        for ct in range(NCT)]
                for s in range(DS):
                    c_sb = bcpool.tile([P, H], BF16, tag="c", bufs=3)
                    nc.scalar.dma_start(c_sb[:], crep[s, :, hs])
                    for ct in range(NCT):
                        dA = work.tile([P, H], F32, tag="dA", bufs=3)
                        nc.scalar.activation(dA[:], delta_t[ct][:, hs], AF.Exp,
                                             scale=acol_t[ct][:, s:s + 1])
                        dBu = work.tile([P, H], BF16, tag="dBu", bufs=3)
                        nc.gpsimd.dma_start(
                            dBu[:], dbus[s, ct * P:(ct + 1) * P, hs])
                        h = work.tile([P, H], BF16, tag="h")
                        init = 0.0 if th == 0 else hstate[ct * DS + s][:]
                        nc.vector.tensor_tensor_scan(
                            h[:], dA[:], dBu[:], init, OP.mult, OP.add)
                        if th == 0:
                            nc.vector.tensor_copy(hstate[ct * DS + s][:],
                                                  h[:, H - 1:H])
                        hc = work.tile([P, H], BF16, tag="hc", bufs=3)
                        nc.vector.tensor_mul(hc[:], h[:], c_sb[:])
                        for nch in range(H // 512):
                            ns = slice(nch * 512, (nch + 1) * 512)
                            nc.tensor.matmul(y_ps[ct][:, ns], ident[:],
                                             hc[:, ns], start=(s == 0),
                                             stop=(s == DS - 1))
                for ct in range(NCT):
                    yD = work.tile([P, H], F32, tag="yD", bufs=1)
                    nc.vector.tensor_add(yD[:], ud_t[ct][:, hs], y_ps[ct][:])
                    silz = work.tile([P, H], F32, tag="silz", bufs=1)
                    nc.scalar.activation(silz[:], z_t[ct][:, hs], AF.Silu)
                    nc.vector.tensor_mul(yz_t[ct][:, hs], yD[:], silz[:])

            # ---- out_proj partials ----
            for mj in range(D // P):
                o_sb = work.tile([P, N], F32, tag="o_sb", bufs=2)
                for nch in range(NTCH):
                    ns = slice(nch * TCH, (nch + 1) * TCH)
                    ps = pspool.tile([P, TCH], F32, tag="big", name="o_ps")
                    for ct in range(NCT):
                        nc.tensor.matmul(ps[:],
                                         wout_t[ct][:, mj * P:(mj + 1) * P],
                                         yz_t[ct][:, ns], start=(ct == 0),
                                         stop=(ct == NCT - 1),
                                         skip_group_check=True)
                    nc.scalar.activation(o_sb[:, ns], ps[:], AF.Copy)
                nc.gpsimd.dma_start(outp[mj * P:(mj + 1) * P, :], o_sb[:])

    nc.compile()
    return nc


_CACHE = {}


def _run_with_retry(nc, in_maps, trace, attempts=3):
    last = None
    for i in range(attempts):
        try:
            return run_bass_kernel_spmd(nc, in_maps,
                                        core_ids=list(range(NCORES)),
                                        trace=trace)
        except Exception as e:  # transient NRT_EXEC_UNIT_UNRECOVERABLE
            last = e
            import time
            time.sleep(2.0 * (i + 1))
    raise last


def _get_programs():
    if "nc1" not in _CACHE:
        _CACHE["nc1"] = _build_launch1()
        _CACHE["nc2"] = _build_launch2()
    return _CACHE["nc1"], _CACHE["nc2"]


# ----------------------------------------------------------------------
# Host orchestration
# ----------------------------------------------------------------------
def kernel(x, Wp, bp, gamma, Win, Wconv, bconv, Wx, Wdt, bdt, A_log,
           Dparam, Wout, _timings=None, _trace=False):
    x = np.asarray(x, np.float32)
    f = lambda a: np.ascontiguousarray(np.asarray(a, np.float32))
    Wp, bp, gamma = f(Wp), f(bp), f(gamma)
    Win, Wconv, bconv = f(Win), f(Wconv), f(bconv)
    Wx, Wdt, bdt = f(Wx), f(Wdt), f(bdt)
    A_log, Dparam, Wout = f(A_log), f(Dparam), f(Wout)

    nc1, nc2 = _get_programs()

    # ---- host prep for launch 1 ----
    prev = np.concatenate([np.zeros((B, 1, D), np.float32), x[:, :-1, :]], 1)
    comb = np.concatenate([x, x - prev], -1)                    # [B,N,2D]
    Wp_aug = np.zeros((KP, D), np.float32)
    Wp_aug[:2 * D, :] = Wp.T
    Wp_aug[2 * D, :] = bp
    Win_eff = (Win * gamma[None, :]).T.copy()                   # [768, 3072]
    Wp_augh = Wp_aug.astype(ml_dtypes.bfloat16)
    Win_effh = Win_eff.astype(ml_dtypes.bfloat16)
    Wxh = Wx.T.copy().astype(ml_dtypes.bfloat16)
    Wdth = Wdt.T.copy().astype(ml_dtypes.bfloat16)
    A = -np.exp(A_log)                                          # [DI, DS]

    # host-computed halo xi (3 tokens before each quarter boundary)
    halo_idx = [TQ - (DC - 1) + j for j in range(DC - 1)]
    in_maps1 = []
    for c in range(NCORES):
        b, q = divmod(c, 4)
        tok = slice(q * TQ, (q + 1) * TQ)
        cT = np.zeros((KP, TQ), np.float32)
        cT[:2 * D, :] = comb[b, tok, :].T
        cT[2 * D, :] = 1.0
        if q == 0:
            xi_halo = np.zeros((DI, DC - 1), np.float32)
        else:
            htok = [q * TQ - (DC - 1) + j for j in range(DC - 1)]
            hc = comb[b, htok, :]                               # [3, 2D]
            hproj = hc @ Wp.T + bp
            hn = hproj * (1.0 / np.sqrt((hproj ** 2).mean(-1, keepdims=True)
                                        + EPS))
            hxz = hn @ Win_eff                                  # [3, 2DI]
            xi_halo = np.ascontiguousarray(hxz[:, :DI].T)       # [DI, 3]
        in_maps1.append({
            "combT": cT.astype(ml_dtypes.bfloat16),
            "wp": Wp_augh, "win": Win_effh,
            "wcb": np.concatenate(
                [Wconv[:, 0, :], bconv[:, None], xi_halo], 1),
            "wx": Wxh, "wdt": Wdth,
        })

    res1 = _run_with_retry(nc1, in_maps1, _trace)
    if _timings is not None:
        _timings.append(res1)

    # ---- host reshard ----
    big = {}
    for b in range(B):
        cs = [b * 4 + q for q in range(4)]
        uT = np.concatenate([res1.results[c]["u_out"] for c in cs],
                            1).astype(np.float32)
        dtl = np.concatenate([res1.results[c]["delta_out"] for c in cs], 1)
        v = dtl + bdt[:, None]
        dT = (np.maximum(v, 0.0) + np.log1p(np.exp(-np.abs(v)))).astype(np.float32)
        zT = np.concatenate([res1.results[c]["z_out"] for c in cs], 1)
        bc = np.concatenate([res1.results[c]["bc_out"] for c in cs], 1)
        duT = dT * uT
        udT = (uT * Dparam[:, None]).astype(np.float32)
        # dbus[s, d, t] = du[d, t] * B[s, t]
        dbus = (duT[None, :, :] *
                bc[:DS, None, :].astype(np.float32)).astype(ml_dtypes.bfloat16)
        crep = np.ascontiguousarray(
            np.broadcast_to(bc[None, DS:, :].transpose(1, 0, 2), (DS, P, N)))
        big[b] = dict(dT=dT, zT=zT, dbus=dbus, udT=udT, crep=crep)

    in_maps2 = []
    for c in range(NCORES):
        b, j = divmod(c, 4)
        dsl = slice(j * DSH, (j + 1) * DSH)
        g = big[b]
        in_maps2.append({
            "delta": np.ascontiguousarray(g["dT"][dsl]),
            "ud": np.ascontiguousarray(g["udT"][dsl]),
            "z": np.ascontiguousarray(g["zT"][dsl]),
            "dbus": np.ascontiguousarray(g["dbus"][:, dsl]),
            "crep": g["crep"],
            "acol": np.ascontiguousarray(A[dsl]),
            "wout": np.ascontiguousarray(Wout[:, dsl].T).astype(
                ml_dtypes.bfloat16),
        })

    res2 = _run_with_retry(nc2, in_maps2, _trace)
    if _timings is not None:
        _timings.append(res2)

    # ---- host gather: sum d-shard partials, add skip ----
    out = np.empty((B, N, D), np.float32)
    for b in range(B):
        acc = res2.results[b * 4]["outp"].copy()
        for j in range(1, 4):
            acc += res2.results[b * 4 + j]["outp"]
        out[b] = acc.T + x[b]
    return out


# revision 23
# speedup vs baseline: 1.0019x; 1.0019x over previous
"""Trainium2 Bass kernel for nn_DeltaVisionMambaBlock.

Self-contained: takes FULL unsharded inputs, returns FULL output.

Decomposition across 8 NeuronCores, two SPMD launches, no collectives:
  Launch 1 (token-sharded: core = batch b x token-quarter q):
    diff-prologue proj (Wp) -> RMSNorm -> in_proj (Win) -> depthwise causal
    conv -> silu -> x_proj (Wx) -> dt_proj (Wdt) -> softplus.
    Channel-transposed layout throughout ([ch, tok]) so no PE transposes.
  Host: reshard [ch, tok] quarters into per-batch [DI, N]; du = delta*u;
    replicate B/C rows across partitions.
  Launch 2 (d_inner-sharded: core = batch b x d-quarter j):
    selective scan via DVE tensor_tensor_scan (time chunks of 512, state
    chained via last-column handoff), y = sum_s (h_s * C_s) accumulated in
    PSUM via identity matmuls, gating, out_proj (Wout) partials.
  Host: sum the 4 d-shard partials per batch, add skip connection.
"""
import sys

if "/opt/trn_rl_repo" not in sys.path:
    sys.path.insert(0, "/opt/trn_rl_repo")

import numpy as np
import ml_dtypes

import concourse.bass as bass
import concourse.tile as tile
from concourse import bacc, mybir
from concourse.bass_utils import run_bass_kernel_spmd
from concourse.masks import make_identity

F32 = mybir.dt.float32
BF16 = mybir.dt.bfloat16
AF = mybir.ActivationFunctionType
OP = mybir.AluOpType

B, N, D = 2, 2048, 768
DI, DS, DC, DTR = 1536, 16, 4, 48
EPS = 1e-5
NCORES = 8
TQ = N // 4            # 512 tokens per launch-1 core
DSH = DI // 4          # 384 d-inner channels per launch-2 core
KP = 13 * 128          # padded contraction dim (2D + bias row -> 1664)
P = 128
TCH = 512              # launch-2 time chunk
NTCH = N // TCH


# ----------------------------------------------------------------------
# Launch 1 program
# ----------------------------------------------------------------------
def _build_launch1():
    nc = bacc.Bacc("TRN2", target_bir_lowering=False, debug=False,
                   num_devices=NCORES)
    combT = nc.dram_tensor("combT", [KP, TQ], BF16, kind="ExternalInput").ap()
    wp = nc.dram_tensor("wp", [KP, D], BF16, kind="ExternalInput").ap()
    win = nc.dram_tensor("win", [D, 2 * DI], BF16, kind="ExternalInput").ap()
    wcb = nc.dram_tensor("wcb", [DI, DC + 1 + DC - 1], F32,
                         kind="ExternalInput").ap()
    wx = nc.dram_tensor("wx", [DI, DTR + 2 * DS], BF16, kind="ExternalInput").ap()
    wdt = nc.dram_tensor("wdt", [DTR, DI], BF16, kind="ExternalInput").ap()

    u_out = nc.dram_tensor("u_out", [DI, TQ], BF16, kind="ExternalOutput").ap()
    delta_out = nc.dram_tensor("delta_out", [DI, TQ], F32, kind="ExternalOutput").ap()
    z_out = nc.dram_tensor("z_out", [DI, TQ], BF16, kind="ExternalOutput").ap()
    bc_out = nc.dram_tensor("bc_out", [2 * DS, TQ], BF16, kind="ExternalOutput").ap()

    NK = KP // P          # 13
    ND = D // P           # 6
    NDI = DI // P         # 12

    with tile.TileContext(nc) as tc:
        with (
            tc.tile_pool(name="const", bufs=1) as cpool,
            tc.tile_pool(name="work", bufs=2) as work,
            tc.tile_pool(name="persist", bufs=1) as persist,
            tc.tile_pool(name="ps", bufs=4, space="PSUM") as pspool,
            tc.tile_pool(name="ps1", bufs=1, space="PSUM") as ps1pool,
        ):
            ones_k = cpool.tile([P, 1], F32, tag="ones_k")
            nc.vector.memset(ones_k[:], 1.0)
            ones_r = cpool.tile([1, P], F32, tag="ones_r")
            nc.vector.memset(ones_r[:], 1.0)

            normed_sb = []
            # ---- stage 1: projT = (Wp_aug.T)^T @ combT, m=channels ----
            with tc.tile_pool(name="s1", bufs=1) as s1:
                combT_sb = []
                wp_sb = []
                for k in range(NK):
                    t = s1.tile([P, TQ], BF16, tag=f"combT{k}")
                    nc.sync.dma_start(t[:], combT[k * P:(k + 1) * P, :])
                    combT_sb.append(t)
                    w = s1.tile([P, D], BF16, tag=f"wp{k}")
                    nc.scalar.dma_start(w[:], wp[k * P:(k + 1) * P, :])
                    wp_sb.append(w)

                proj_sb = []
                sq_sb = []
                for m in range(ND):
                    ps = pspool.tile([P, TQ], F32, tag="mm_ps")
                    for k in range(NK):
                        nc.tensor.matmul(ps[:], wp_sb[k][:, m * P:(m + 1) * P],
                                         combT_sb[k][:], start=(k == 0),
                                         stop=(k == NK - 1))
                    p_sb = s1.tile([P, TQ], F32, tag=f"proj{m}")
                    nc.vector.tensor_copy(p_sb[:], ps[:])
                    proj_sb.append(p_sb)
                    s_sb = s1.tile([P, TQ], F32, tag=f"sq{m}")
                    nc.scalar.activation(s_sb[:], ps[:], AF.Square)
                    sq_sb.append(s_sb)

                ssq_ps = ps1pool.tile([1, TQ], F32, tag="ssq")
                for m in range(ND):
                    nc.tensor.matmul(ssq_ps[:], ones_k[:], sq_sb[m][:],
                                     start=(m == 0), stop=(m == ND - 1))
                eps_t = cpool.tile([1, 1], F32, tag="eps")
                nc.vector.memset(eps_t[:], EPS)
                sqrt_sb = work.tile([1, TQ], F32, tag="sqrt")
                nc.scalar.activation(sqrt_sb[:], ssq_ps[:], AF.Sqrt,
                                     bias=eps_t[:, 0:1], scale=1.0 / D)
                rms_sb = work.tile([1, TQ], F32, tag="rms")
                nc.vector.reciprocal(rms_sb[:], sqrt_sb[:])
                rms_ps = ps1pool.tile([P, TQ], F32, tag="rmsb")
                nc.tensor.matmul(rms_ps[:], ones_r[:], rms_sb[:],
                                 start=True, stop=True)

                for m in range(ND):
                    nt = persist.tile([P, TQ], BF16, tag=f"normed{m}")
                    nc.vector.tensor_mul(nt[:], proj_sb[m][:], rms_ps[:])
                    normed_sb.append(nt)

            # ---- stage 2: xzT = (Win_eff.T)^T @ normedT ----
            xi_ext = []
            wcb_t = []
            for mi in range(NDI):
                wc = cpool.tile([P, DC + 1], F32, tag=f"wcb{mi}")
                nc.sync.dma_start(wc[:], wcb[mi * P:(mi + 1) * P, 0:DC + 1])
                wcb_t.append(wc)
                xe = persist.tile([P, (DC - 1) + TQ], F32, tag=f"xiext{mi}")
                nc.sync.dma_start(xe[:, 0:DC - 1],
                                  wcb[mi * P:(mi + 1) * P, DC + 1:])
                xi_ext.append(xe)

            with tc.tile_pool(name="s2", bufs=1) as s2:
                win_sb = []
                for k in range(ND):
                    w = s2.tile([P, 2 * DI], BF16, tag=f"win{k}")
                    nc.scalar.dma_start(w[:], win[k * P:(k + 1) * P, :])
                    win_sb.append(w)

                for mi in range(2 * NDI):
                    ps = pspool.tile([P, TQ], F32, tag="mm_ps")
                    for k in range(ND):
                        nc.tensor.matmul(ps[:],
                                         win_sb[k][:, mi * P:(mi + 1) * P],
                                         normed_sb[k][:], start=(k == 0),
                                         stop=(k == ND - 1))
                    if mi < NDI:
                        nc.scalar.activation(xi_ext[mi][:, DC - 1:], ps[:],
                                             AF.Copy)
                    else:
                        z_sb = work.tile([P, TQ], BF16, tag="z")
                        nc.scalar.activation(z_sb[:], ps[:], AF.Copy)
                        j = mi - NDI
                        nc.gpsimd.dma_start(z_out[j * P:(j + 1) * P, :], z_sb[:])

            # ---- conv + silu -> u ----
            u_sb = []
            for mi in range(NDI):
                acc = work.tile([P, TQ], F32, tag="cacc")
                nc.vector.tensor_scalar(acc[:], xi_ext[mi][:, 0:TQ],
                                        wcb_t[mi][:, 0:1], None, OP.mult)
                for j in range(1, DC):
                    acc2 = work.tile([P, TQ], F32, tag="cacc")
                    nc.vector.scalar_tensor_tensor(
                        acc2[:], xi_ext[mi][:, j:j + TQ], wcb_t[mi][:, j:j + 1],
                        acc[:], OP.mult, OP.add)
                    acc = acc2
                ut = persist.tile([P, TQ], BF16, tag=f"u{mi}")
                nc.scalar.activation(ut[:], acc[:], AF.Silu,
                                     bias=wcb_t[mi][:, DC:DC + 1])
                u_sb.append(ut)
                nc.gpsimd.dma_start(u_out[mi * P:(mi + 1) * P, :], ut[:])

            # ---- x_dbl = (Wx.T)^T @ uT ----
            wx_sb = []
            for k in range(NDI):
                w = cpool.tile([P, DTR + 2 * DS], BF16, tag=f"wx{k}")
                nc.sync.dma_start(w[:], wx[k * P:(k + 1) * P, :])
                wx_sb.append(w)
            xdbl_ps = ps1pool.tile([DTR + 2 * DS, TQ], F32, tag="xdbl")
            for k in range(NDI):
                nc.tensor.matmul(xdbl_ps[:], wx_sb[k][:], u_sb[k][:],
                                 start=(k == 0), stop=(k == NDI - 1))
            xdbl_sb = persist.tile([DTR + 2 * DS, TQ], BF16, tag="xdbl_sb")
            nc.scalar.activation(xdbl_sb[:], xdbl_ps[:], AF.Copy)
            nc.sync.dma_start(bc_out[:], xdbl_sb[DTR:, :])

            # ---- dt_lin = Wdt @ dt (softplus+bias on host) ----
            wdt_sb = cpool.tile([DTR, DI], BF16, tag="wdt")
            nc.sync.dma_start(wdt_sb[:], wdt[:])
            for mi in range(NDI):
                ps = pspool.tile([P, TQ], F32, tag="mm_ps")
                nc.tensor.matmul(ps[:], wdt_sb[:, mi * P:(mi + 1) * P],
                                 xdbl_sb[0:DTR, :], start=True, stop=True)
                d_sb = work.tile([P, TQ], F32, tag="delta")
                nc.scalar.activation(d_sb[:], ps[:], AF.Copy)
                nc.gpsimd.dma_start(delta_out[mi * P:(mi + 1) * P, :], d_sb[:])

    nc.compile()
    return nc


# ----------------------------------------------------------------------
# Launch 2 program
# ----------------------------------------------------------------------
def _build_launch2():
    nc = bacc.Bacc("TRN2", target_bir_lowering=False, debug=False,
                   num_devices=NCORES)
    delta = nc.dram_tensor("delta", [DSH, N], F32, kind="ExternalInput").ap()
    ud = nc.dram_tensor("ud", [DSH, N], F32, kind="ExternalInput").ap()
    z = nc.dram_tensor("z", [DSH, N], BF16, kind="ExternalInput").ap()
    dbus = nc.dram_tensor("dbus", [DS, DSH, N], BF16, kind="ExternalInput").ap()
    crep = nc.dram_tensor("crep", [DS, P, N], BF16, kind="ExternalInput").ap()
    acol = nc.dram_tensor("acol", [DSH, DS], F32, kind="ExternalInput").ap()
    wout = nc.dram_tensor("wout", [DSH, D], BF16, kind="ExternalInput").ap()
    outp = nc.dram_tensor("outp", [D, N], F32, kind="ExternalOutput").ap()

    NCT = DSH // P        # 3

    with tile.TileContext(nc) as tc:
        with (
            tc.tile_pool(name="const", bufs=1) as cpool,
            tc.tile_pool(name="inp", bufs=1) as inp,
            tc.tile_pool(name="bc", bufs=2) as bcpool,
            tc.tile_pool(name="work", bufs=2) as work,
            tc.tile_pool(name="yz", bufs=1) as yzpool,
            tc.tile_pool(name="ps", bufs=2, space="PSUM") as pspool,
        ):
            ident = cpool.tile([P, P], BF16, tag="ident")
            make_identity(nc, ident[:])

            delta_t, ud_t, z_t, acol_t, wout_t = [], [], [], [], []
            for ct in range(NCT):
                r = slice(ct * P, (ct + 1) * P)
                t = inp.tile([P, N], F32, tag=f"dlt{ct}")
                nc.sync.dma_start(t[:], delta[r, :])
                delta_t.append(t)
                t = inp.tile([P, N], F32, tag=f"ud{ct}")
                nc.sync.dma_start(t[:], ud[r, :])
                ud_t.append(t)
                t = inp.tile([P, N], BF16, tag=f"z{ct}")
                nc.sync.dma_start(t[:], z[r, :])
                z_t.append(t)
                t = inp.tile([P, DS], F32, tag=f"acol{ct}")
                nc.sync.dma_start(t[:], acol[r, :])
                acol_t.append(t)
                t = inp.tile([P, D], BF16, tag=f"wout{ct}")
                nc.sync.dma_start(t[:], wout[r, :])
                wout_t.append(t)
            yz_t = [yzpool.tile([P, N], BF16, tag=f"yz{ct}", name=f"yz{ct}")
                    for ct in range(NCT)]

            # ct passes: {0,1} then {2} -- two 2048-wide psum accumulators max
            for cts in ([0, 1], [2]):
                y_ps = {}
                for ct in cts:
                    y_ps[ct] = pspool.tile([P, N], F32, tag="big",
                                           name=f"yps{ct}")
                for s in range(DS):
                    c_sb = bcpool.tile([P, N], BF16, tag="c", bufs=3)
                    nc.scalar.dma_start(c_sb[:], crep[s, :, :])
                    for ct in cts:
                        dA = work.tile([P, N], F32, tag="dA", bufs=3)
                        nc.scalar.activation(dA[:], delta_t[ct][:], AF.Exp,
                                             scale=acol_t[ct][:, s:s + 1])
                        dBu = work.tile([P, N], BF16, tag="dBu", bufs=3)
                        nc.gpsimd.dma_start(
                            dBu[:], dbus[s, ct * P:(ct + 1) * P, :])
                        h = work.tile([P, N], BF16, tag="h")
                        nc.vector.tensor_tensor_scan(
                            h[:], dA[:], dBu[:], 0.0, OP.mult, OP.add)
                        hc = work.tile([P, N], BF16, tag="hc", bufs=3)
                        nc.vector.tensor_mul(hc[:], h[:], c_sb[:])
                        for nch in range(N // 512):
                            ns = slice(nch * 512, (nch + 1) * 512)
                            nc.tensor.matmul(y_ps[ct][:, ns], ident[:],
                                             hc[:, ns], start=(s == 0),
                                             stop=(s == DS - 1))
                for ct in cts:
                    yD = work.tile([P, N], F32, tag="yD", bufs=1)
                    nc.vector.tensor_add(yD[:], ud_t[ct][:], y_ps[ct][:])
                    silz = work.tile([P, N], F32, tag="silz", bufs=1)
                    nc.scalar.activation(silz[:], z_t[ct][:], AF.Silu)
                    nc.vector.tensor_mul(yz_t[ct][:], yD[:], silz[:])

            # ---- out_proj partials ----
            for mj in range(D // P):
                o_sb = work.tile([P, N], F32, tag="o_sb", bufs=2)
                for nch in range(NTCH):
                    ns = slice(nch * TCH, (nch + 1) * TCH)
                    ps = pspool.tile([P, TCH], F32, tag="big", name="o_ps")
                    for ct in range(NCT):
                        nc.tensor.matmul(ps[:],
                                         wout_t[ct][:, mj * P:(mj + 1) * P],
                                         yz_t[ct][:, ns], start=(ct == 0),
                                         stop=(ct == NCT - 1),
                                         skip_group_check=True)
                    nc.scalar.activation(o_sb[:, ns], ps[:], AF.Copy)
                nc.gpsimd.dma_start(outp[mj * P:(mj + 1) * P, :], o_sb[:])

    nc.compile()
    return nc


_CACHE = {}


def _run_with_retry(nc, in_maps, trace, attempts=3):
    last = None
    for i in range(attempts):
        try:
            return run_bass_kernel_spmd(nc, in_maps,
                                        core_ids=list(range(NCORES)),
                                        trace=trace)
        except Exception as e:  # transient NRT_EXEC_UNIT_UNRECOVERABLE
            last = e
            import time
            time.sleep(2.0 * (i + 1))
    raise last


def _get_programs():
    if "nc1" not in _CACHE:
        _CACHE["nc1"] = _build_launch1()
        _CACHE["nc2"] = _build_launch2()
    return _CACHE["nc1"], _CACHE["nc2"]


# ----------------------------------------------------------------------
# Host orchestration
# ----------------------------------------------------------------------
def kernel(x, Wp, bp, gamma, Win, Wconv, bconv, Wx, Wdt, bdt, A_log,
           Dparam, Wout, _timings=None, _trace=False):
    x = np.asarray(x, np.float32)
    f = lambda a: np.ascontiguousarray(np.asarray(a, np.float32))
    Wp, bp, gamma = f(Wp), f(bp), f(gamma)
    Win, Wconv, bconv = f(Win), f(Wconv), f(bconv)
    Wx, Wdt, bdt = f(Wx), f(Wdt), f(bdt)
    A_log, Dparam, Wout = f(A_log), f(Dparam), f(Wout)

    nc1, nc2 = _get_programs()

    # ---- host prep for launch 1 ----
    prev = np.concatenate([np.zeros((B, 1, D), np.float32), x[:, :-1, :]], 1)
    comb = np.concatenate([x, x - prev], -1)                    # [B,N,2D]
    Wp_aug = np.zeros((KP, D), np.float32)
    Wp_aug[:2 * D, :] = Wp.T
    Wp_aug[2 * D, :] = bp
    Win_eff = (Win * gamma[None, :]).T.copy()                   # [768, 3072]
    Wp_augh = Wp_aug.astype(ml_dtypes.bfloat16)
    Win_effh = Win_eff.astype(ml_dtypes.bfloat16)
    Wxh = Wx.T.copy().astype(ml_dtypes.bfloat16)
    Wdth = Wdt.T.copy().astype(ml_dtypes.bfloat16)
    A = -np.exp(A_log)                                          # [DI, DS]

    # host-computed halo xi (3 tokens before each quarter boundary)
    halo_idx = [TQ - (DC - 1) + j for j in range(DC - 1)]
    in_maps1 = []
    for c in range(NCORES):
        b, q = divmod(c, 4)
        tok = slice(q * TQ, (q + 1) * TQ)
        cT = np.zeros((KP, TQ), np.float32)
        cT[:2 * D, :] = comb[b, tok, :].T
        cT[2 * D, :] = 1.0
        if q == 0:
            xi_halo = np.zeros((DI, DC - 1), np.float32)
        else:
            htok = [q * TQ - (DC - 1) + j for j in range(DC - 1)]
            hc = comb[b, htok, :]                               # [3, 2D]
            hproj = hc @ Wp.T + bp
            hn = hproj * (1.0 / np.sqrt((hproj ** 2).mean(-1, keepdims=True)
                                        + EPS))
            hxz = hn @ Win_eff                                  # [3, 2DI]
            xi_halo = np.ascontiguousarray(hxz[:, :DI].T)       # [DI, 3]
        in_maps1.append({
            "combT": cT.astype(ml_dtypes.bfloat16),
            "wp": Wp_augh, "win": Win_effh,
            "wcb": np.concatenate(
                [Wconv[:, 0, :], bconv[:, None], xi_halo], 1),
            "wx": Wxh, "wdt": Wdth,
        })

    res1 = _run_with_retry(nc1, in_maps1, _trace)
    if _timings is not None:
        _timings.append(res1)

    # ---- host reshard ----
    big = {}
    for b in range(B):
        cs = [b * 4 + q for q in range(4)]
        uT = np.concatenate([res1.results[c]["u_out"] for c in cs],
                            1).astype(np.float32)
        dtl = np.concatenate([res1.results[c]["delta_out"] for c in cs], 1)
        v = dtl + bdt[:, None]
        dT = (np.maximum(v, 0.0) + np.log1p(np.exp(-np.abs(v)))).astype(np.float32)
        zT = np.concatenate([res1.results[c]["z_out"] for c in cs], 1)
        bc = np.concatenate([res1.results[c]["bc_out"] for c in cs], 1)
        duT = dT * uT
        udT = (uT * Dparam[:, None]).astype(np.float32)
        # dbus[s, d, t] = du[d, t] * B[s, t]
        dbus = (duT[None, :, :] *
                bc[:DS, None, :].astype(np.float32)).astype(ml_dtypes.bfloat16)
        crep = np.ascontiguousarray(
            np.broadcast_to(bc[None, DS:, :].transpose(1, 0, 2), (DS, P, N)))
        big[b] = dict(dT=dT, zT=zT, dbus=dbus, udT=udT, crep=crep)

    in_maps2 = []
    for c in range(NCORES):
        b, j = divmod(c, 4)
        dsl = slice(j * DSH, (j + 1) * DSH)
        g = big[b]
        in_maps2.append({
            "delta": np.ascontiguousarray(g["dT"][dsl]),
            "ud": np.ascontiguousarray(g["udT"][dsl]),
            "z": np.ascontiguousarray(g["zT"][dsl]),
            "dbus": np.ascontiguousarray(g["dbus"][:, dsl]),
            "crep": g["crep"],
            "acol": np.ascontiguousarray(A[dsl]),
            "wout": np.ascontiguousarray(Wout[:, dsl].T).astype(
                ml_dtypes.bfloat16),
        })

    res2 = _run_with_retry(nc2, in_maps2, _trace)
    if _timings is not None:
        _timings.append(res2)

    # ---- host gather: sum d-shard partials, add skip ----
    out = np.empty((B, N, D), np.float32)
    for b in range(B):
        acc = res2.results[b * 4]["outp"].copy()
        for j in range(1, 4):
            acc += res2.results[b * 4 + j]["outp"]
        out[b] = acc.T + x[b]
    return out


# revision 24
# speedup vs baseline: 1.0064x; 1.0045x over previous
"""Trainium2 Bass kernel for nn_DeltaVisionMambaBlock.

Self-contained: takes FULL unsharded inputs, returns FULL output.

Decomposition across 8 NeuronCores, two SPMD launches, no collectives:
  Launch 1 (token-sharded: core = batch b x token-quarter q):
    diff-prologue proj (Wp) -> RMSNorm -> in_proj (Win) -> depthwise causal
    conv -> silu -> x_proj (Wx) -> dt_proj (Wdt) -> softplus.
    Channel-transposed layout throughout ([ch, tok]) so no PE transposes.
  Host: reshard [ch, tok] quarters into per-batch [DI, N]; du = delta*u;
    replicate B/C rows across partitions.
  Launch 2 (d_inner-sharded: core = batch b x d-quarter j):
    selective scan via DVE tensor_tensor_scan (time chunks of 512, state
    chained via last-column handoff), y = sum_s (h_s * C_s) accumulated in
    PSUM via identity matmuls, gating, out_proj (Wout) partials.
  Host: sum the 4 d-shard partials per batch, add skip connection.
"""
import sys

if "/opt/trn_rl_repo" not in sys.path:
    sys.path.insert(0, "/opt/trn_rl_repo")

import numpy as np
import ml_dtypes

import concourse.bass as bass
import concourse.tile as tile
from concourse import bacc, mybir
from concourse.bass_utils import run_bass_kernel_spmd
from concourse.masks import make_identity

F32 = mybir.dt.float32
BF16 = mybir.dt.bfloat16
AF = mybir.ActivationFunctionType
OP = mybir.AluOpType

B, N, D = 2, 2048, 768
DI, DS, DC, DTR = 1536, 16, 4, 48
EPS = 1e-5
NCORES = 8
TQ = N // 4            # 512 tokens per launch-1 core
DSH = DI // 4          # 384 d-inner channels per launch-2 core
KP = 13 * 128          # padded contraction dim (2D + bias row -> 1664)
P = 128
TCH = 512              # launch-2 time chunk
NTCH = N // TCH


# ----------------------------------------------------------------------
# Launch 1 program
# ----------------------------------------------------------------------
def _build_launch1():
    nc = bacc.Bacc("TRN2", target_bir_lowering=False, debug=False,
                   num_devices=NCORES)
    combT = nc.dram_tensor("combT", [KP, TQ], BF16, kind="ExternalInput").ap()
    wp = nc.dram_tensor("wp", [KP, D], BF16, kind="ExternalInput").ap()
    win = nc.dram_tensor("win", [D, 2 * DI], BF16, kind="ExternalInput").ap()
    wcb = nc.dram_tensor("wcb", [DI, DC + 1 + DC - 1], F32,
                         kind="ExternalInput").ap()
    wx = nc.dram_tensor("wx", [DI, DTR + 2 * DS], BF16, kind="ExternalInput").ap()
    wdt = nc.dram_tensor("wdt", [DTR, DI], BF16, kind="ExternalInput").ap()

    u_out = nc.dram_tensor("u_out", [DI, TQ], BF16, kind="ExternalOutput").ap()
    delta_out = nc.dram_tensor("delta_out", [DI, TQ], F32, kind="ExternalOutput").ap()
    z_out = nc.dram_tensor("z_out", [DI, TQ], BF16, kind="ExternalOutput").ap()
    bc_out = nc.dram_tensor("bc_out", [2 * DS, TQ], BF16, kind="ExternalOutput").ap()

    NK = KP // P          # 13
    ND = D // P           # 6
    NDI = DI // P         # 12

    with tile.TileContext(nc) as tc:
        with (
            tc.tile_pool(name="const", bufs=1) as cpool,
            tc.tile_pool(name="work", bufs=2) as work,
            tc.tile_pool(name="persist", bufs=1) as persist,
            tc.tile_pool(name="ps", bufs=4, space="PSUM") as pspool,
            tc.tile_pool(name="ps1", bufs=1, space="PSUM") as ps1pool,
        ):
            ones_k = cpool.tile([P, 1], F32, tag="ones_k")
            nc.vector.memset(ones_k[:], 1.0)
            ones_r = cpool.tile([1, P], F32, tag="ones_r")
            nc.vector.memset(ones_r[:], 1.0)

            normed_sb = []
            # ---- stage 1: projT = (Wp_aug.T)^T @ combT, m=channels ----
            with tc.tile_pool(name="s1", bufs=1) as s1:
                combT_sb = []
                wp_sb = []
                for k in range(NK):
                    t = s1.tile([P, TQ], BF16, tag=f"combT{k}")
                    nc.sync.dma_start(t[:], combT[k * P:(k + 1) * P, :])
                    combT_sb.append(t)
                    w = s1.tile([P, D], BF16, tag=f"wp{k}")
                    nc.scalar.dma_start(w[:], wp[k * P:(k + 1) * P, :])
                    wp_sb.append(w)

                proj_sb = []
                sq_sb = []
                for m in range(ND):
                    ps = pspool.tile([P, TQ], F32, tag="mm_ps")
                    for k in range(NK):
                        nc.tensor.matmul(ps[:], wp_sb[k][:, m * P:(m + 1) * P],
                                         combT_sb[k][:], start=(k == 0),
                                         stop=(k == NK - 1))
                    p_sb = persist.tile([P, TQ], BF16, tag=f"proj{m}")
                    nc.vector.tensor_copy(p_sb[:], ps[:])
                    proj_sb.append(p_sb)
                    s_sb = s1.tile([P, TQ], F32, tag=f"sq{m}")
                    nc.scalar.activation(s_sb[:], ps[:], AF.Square)
                    sq_sb.append(s_sb)

                ssq_ps = ps1pool.tile([1, TQ], F32, tag="ssq")
                for m in range(ND):
                    nc.tensor.matmul(ssq_ps[:], ones_k[:], sq_sb[m][:],
                                     start=(m == 0), stop=(m == ND - 1))
                eps_t = cpool.tile([1, 1], F32, tag="eps")
                nc.vector.memset(eps_t[:], EPS)
                sqrt_sb = work.tile([1, TQ], F32, tag="sqrt")
                nc.scalar.activation(sqrt_sb[:], ssq_ps[:], AF.Sqrt,
                                     bias=eps_t[:, 0:1], scale=1.0 / D)
                rms_sb = work.tile([1, TQ], F32, tag="rms")
                nc.vector.reciprocal(rms_sb[:], sqrt_sb[:])
                rms_ps = ps1pool.tile([P, TQ], F32, tag="rmsb")
                nc.tensor.matmul(rms_ps[:], ones_r[:], rms_sb[:],
                                 start=True, stop=True)
                rms_bc = persist.tile([P, TQ], F32, tag="rms_bc")
                nc.vector.tensor_copy(rms_bc[:], rms_ps[:])
                normed_sb = proj_sb

            # ---- stage 2: xzT = (Win_eff.T)^T @ normedT ----
            xi_ext = []
            wcb_t = []
            for mi in range(NDI):
                wc = cpool.tile([P, DC + 1], F32, tag=f"wcb{mi}")
                nc.sync.dma_start(wc[:], wcb[mi * P:(mi + 1) * P, 0:DC + 1])
                wcb_t.append(wc)
                xe = persist.tile([P, (DC - 1) + TQ], F32, tag=f"xiext{mi}")
                nc.sync.dma_start(xe[:, 0:DC - 1],
                                  wcb[mi * P:(mi + 1) * P, DC + 1:])
                xi_ext.append(xe)

            with tc.tile_pool(name="s2", bufs=1) as s2:
                win_sb = []
                for k in range(ND):
                    w = s2.tile([P, 2 * DI], BF16, tag=f"win{k}")
                    nc.scalar.dma_start(w[:], win[k * P:(k + 1) * P, :])
                    win_sb.append(w)

                for mi in range(2 * NDI):
                    ps = pspool.tile([P, TQ], F32, tag="mm_ps")
                    for k in range(ND):
                        nc.tensor.matmul(ps[:],
                                         win_sb[k][:, mi * P:(mi + 1) * P],
                                         normed_sb[k][:], start=(k == 0),
                                         stop=(k == ND - 1))
                    if mi < NDI:
                        nc.vector.tensor_mul(xi_ext[mi][:, DC - 1:], ps[:],
                                             rms_bc[:])
                    else:
                        z_sb = work.tile([P, TQ], BF16, tag="z")
                        nc.vector.tensor_mul(z_sb[:], ps[:], rms_bc[:])
                        j = mi - NDI
                        nc.gpsimd.dma_start(z_out[j * P:(j + 1) * P, :], z_sb[:])

            # ---- conv + silu -> u ----
            u_sb = []
            for mi in range(NDI):
                acc = work.tile([P, TQ], F32, tag="cacc")
                nc.vector.tensor_scalar(acc[:], xi_ext[mi][:, 0:TQ],
                                        wcb_t[mi][:, 0:1], None, OP.mult)
                for j in range(1, DC):
                    acc2 = work.tile([P, TQ], F32, tag="cacc")
                    nc.vector.scalar_tensor_tensor(
                        acc2[:], xi_ext[mi][:, j:j + TQ], wcb_t[mi][:, j:j + 1],
                        acc[:], OP.mult, OP.add)
                    acc = acc2
                ut = persist.tile([P, TQ], BF16, tag=f"u{mi}")
                nc.scalar.activation(ut[:], acc[:], AF.Silu,
                                     bias=wcb_t[mi][:, DC:DC + 1])
                u_sb.append(ut)
                nc.gpsimd.dma_start(u_out[mi * P:(mi + 1) * P, :], ut[:])

            # ---- x_dbl = (Wx.T)^T @ uT ----
            wx_sb = []
            for k in range(NDI):
                w = cpool.tile([P, DTR + 2 * DS], BF16, tag=f"wx{k}")
                nc.sync.dma_start(w[:], wx[k * P:(k + 1) * P, :])
                wx_sb.append(w)
            xdbl_ps = ps1pool.tile([DTR + 2 * DS, TQ], F32, tag="xdbl")
            for k in range(NDI):
                nc.tensor.matmul(xdbl_ps[:], wx_sb[k][:], u_sb[k][:],
                                 start=(k == 0), stop=(k == NDI - 1))
            xdbl_sb = persist.tile([DTR + 2 * DS, TQ], BF16, tag="xdbl_sb")
            nc.scalar.activation(xdbl_sb[:], xdbl_ps[:], AF.Copy)
            nc.sync.dma_start(bc_out[:], xdbl_sb[DTR:, :])

            # ---- dt_lin = Wdt @ dt (softplus+bias on host) ----
            wdt_sb = cpool.tile([DTR, DI], BF16, tag="wdt")
            nc.sync.dma_start(wdt_sb[:], wdt[:])
            for mi in range(NDI):
                ps = pspool.tile([P, TQ], F32, tag="mm_ps")
                nc.tensor.matmul(ps[:], wdt_sb[:, mi * P:(mi + 1) * P],
                                 xdbl_sb[0:DTR, :], start=True, stop=True)
                d_sb = work.tile([P, TQ], F32, tag="delta")
                nc.scalar.activation(d_sb[:], ps[:], AF.Copy)
                nc.gpsimd.dma_start(delta_out[mi * P:(mi + 1) * P, :], d_sb[:])

    nc.compile()
    return nc


# ----------------------------------------------------------------------
# Launch 2 program
# ----------------------------------------------------------------------
def _build_launch2():
    nc = bacc.Bacc("TRN2", target_bir_lowering=False, debug=False,
                   num_devices=NCORES)
    delta = nc.dram_tensor("delta", [DSH, N], F32, kind="ExternalInput").ap()
    ud = nc.dram_tensor("ud", [DSH, N], F32, kind="ExternalInput").ap()
    z = nc.dram_tensor("z", [DSH, N], BF16, kind="ExternalInput").ap()
    dbus = nc.dram_tensor("dbus", [DS, DSH, N], BF16, kind="ExternalInput").ap()
    crep = nc.dram_tensor("crep", [DS, P, N], BF16, kind="ExternalInput").ap()
    acol = nc.dram_tensor("acol", [DSH, DS], F32, kind="ExternalInput").ap()
    wout = nc.dram_tensor("wout", [DSH, D], BF16, kind="ExternalInput").ap()
    outp = nc.dram_tensor("outp", [D, N], F32, kind="ExternalOutput").ap()

    NCT = DSH // P        # 3

    with tile.TileContext(nc) as tc:
        with (
            tc.tile_pool(name="const", bufs=1) as cpool,
            tc.tile_pool(name="inp", bufs=1) as inp,
            tc.tile_pool(name="bc", bufs=2) as bcpool,
            tc.tile_pool(name="work", bufs=2) as work,
            tc.tile_pool(name="yz", bufs=1) as yzpool,
            tc.tile_pool(name="ps", bufs=2, space="PSUM") as pspool,
        ):
            ident = cpool.tile([P, P], BF16, tag="ident")
            make_identity(nc, ident[:])

            delta_t, ud_t, z_t, acol_t, wout_t = [], [], [], [], []
            for ct in range(NCT):
                r = slice(ct * P, (ct + 1) * P)
                t = inp.tile([P, N], F32, tag=f"dlt{ct}")
                nc.sync.dma_start(t[:], delta[r, :])
                delta_t.append(t)
                t = inp.tile([P, N], F32, tag=f"ud{ct}")
                nc.sync.dma_start(t[:], ud[r, :])
                ud_t.append(t)
                t = inp.tile([P, N], BF16, tag=f"z{ct}")
                nc.sync.dma_start(t[:], z[r, :])
                z_t.append(t)
                t = inp.tile([P, DS], F32, tag=f"acol{ct}")
                nc.sync.dma_start(t[:], acol[r, :])
                acol_t.append(t)
                t = inp.tile([P, D], BF16, tag=f"wout{ct}")
                nc.sync.dma_start(t[:], wout[r, :])
                wout_t.append(t)
            yz_t = [yzpool.tile([P, N], BF16, tag=f"yz{ct}", name=f"yz{ct}")
                    for ct in range(NCT)]

            # ct passes: {0,1} then {2} -- two 2048-wide psum accumulators max
            for cts in ([0, 1], [2]):
                y_ps = {}
                for ct in cts:
                    y_ps[ct] = pspool.tile([P, N], F32, tag="big",
                                           name=f"yps{ct}")
                for s in range(DS):
                    c_sb = bcpool.tile([P, N], BF16, tag="c", bufs=3)
                    nc.scalar.dma_start(c_sb[:], crep[s, :, :])
                    for ct in cts:
                        dA = work.tile([P, N], F32, tag="dA", bufs=3)
                        nc.scalar.activation(dA[:], delta_t[ct][:], AF.Exp,
                                             scale=acol_t[ct][:, s:s + 1])
                        dBu = work.tile([P, N], BF16, tag="dBu", bufs=3)
                        nc.gpsimd.dma_start(
                            dBu[:], dbus[s, ct * P:(ct + 1) * P, :])
                        h = work.tile([P, N], BF16, tag="h")
                        nc.vector.tensor_tensor_scan(
                            h[:], dA[:], dBu[:], 0.0, OP.mult, OP.add)
                        hc = work.tile([P, N], BF16, tag="hc", bufs=3)
                        nc.vector.tensor_mul(hc[:], h[:], c_sb[:])
                        for nch in range(N // 512):
                            ns = slice(nch * 512, (nch + 1) * 512)
                            nc.tensor.matmul(y_ps[ct][:, ns], ident[:],
                                             hc[:, ns], start=(s == 0),
                                             stop=(s == DS - 1))
                for ct in cts:
                    yD = work.tile([P, N], F32, tag="yD", bufs=1)
                    nc.vector.tensor_add(yD[:], ud_t[ct][:], y_ps[ct][:])
                    silz = work.tile([P, N], F32, tag="silz", bufs=1)
                    nc.scalar.activation(silz[:], z_t[ct][:], AF.Silu)
                    nc.vector.tensor_mul(yz_t[ct][:], yD[:], silz[:])

            # ---- out_proj partials ----
            for mj in range(D // P):
                o_sb = work.tile([P, N], F32, tag="o_sb", bufs=2)
                for nch in range(NTCH):
                    ns = slice(nch * TCH, (nch + 1) * TCH)
                    ps = pspool.tile([P, TCH], F32, tag="big", name="o_ps")
                    for ct in range(NCT):
                        nc.tensor.matmul(ps[:],
                                         wout_t[ct][:, mj * P:(mj + 1) * P],
                                         yz_t[ct][:, ns], start=(ct == 0),
                                         stop=(ct == NCT - 1),
                                         skip_group_check=True)
                    nc.scalar.activation(o_sb[:, ns], ps[:], AF.Copy)
                nc.gpsimd.dma_start(outp[mj * P:(mj + 1) * P, :], o_sb[:])

    nc.compile()
    return nc


_CACHE = {}


def _run_with_retry(nc, in_maps, trace, attempts=3):
    last = None
    for i in range(attempts):
        try:
            return run_bass_kernel_spmd(nc, in_maps,
                                        core_ids=list(range(NCORES)),
                                        trace=trace)
        except Exception as e:  # transient NRT_EXEC_UNIT_UNRECOVERABLE
            last = e
            import time
            time.sleep(2.0 * (i + 1))
    raise last


def _get_programs():
    if "nc1" not in _CACHE:
        _CACHE["nc1"] = _build_launch1()
        _CACHE["nc2"] = _build_launch2()
    return _CACHE["nc1"], _CACHE["nc2"]


# ----------------------------------------------------------------------
# Host orchestration
# ----------------------------------------------------------------------
def kernel(x, Wp, bp, gamma, Win, Wconv, bconv, Wx, Wdt, bdt, A_log,
           Dparam, Wout, _timings=None, _trace=False):
    x = np.asarray(x, np.float32)
    f = lambda a: np.ascontiguousarray(np.asarray(a, np.float32))
    Wp, bp, gamma = f(Wp), f(bp), f(gamma)
    Win, Wconv, bconv = f(Win), f(Wconv), f(bconv)
    Wx, Wdt, bdt = f(Wx), f(Wdt), f(bdt)
    A_log, Dparam, Wout = f(A_log), f(Dparam), f(Wout)

    nc1, nc2 = _get_programs()

    # ---- host prep for launch 1 ----
    prev = np.concatenate([np.zeros((B, 1, D), np.float32), x[:, :-1, :]], 1)
    comb = np.concatenate([x, x - prev], -1)                    # [B,N,2D]
    Wp_aug = np.zeros((KP, D), np.float32)
    Wp_aug[:2 * D, :] = Wp.T
    Wp_aug[2 * D, :] = bp
    Win_eff = (Win * gamma[None, :]).T.copy()                   # [768, 3072]
    Wp_augh = Wp_aug.astype(ml_dtypes.bfloat16)
    Win_effh = Win_eff.astype(ml_dtypes.bfloat16)
    Wxh = Wx.T.copy().astype(ml_dtypes.bfloat16)
    Wdth = Wdt.T.copy().astype(ml_dtypes.bfloat16)
    A = -np.exp(A_log)                                          # [DI, DS]

    # host-computed halo xi (3 tokens before each quarter boundary)
    halo_idx = [TQ - (DC - 1) + j for j in range(DC - 1)]
    in_maps1 = []
    for c in range(NCORES):
        b, q = divmod(c, 4)
        tok = slice(q * TQ, (q + 1) * TQ)
        cT = np.zeros((KP, TQ), np.float32)
        cT[:2 * D, :] = comb[b, tok, :].T
        cT[2 * D, :] = 1.0
        if q == 0:
            xi_halo = np.zeros((DI, DC - 1), np.float32)
        else:
            htok = [q * TQ - (DC - 1) + j for j in range(DC - 1)]
            hc = comb[b, htok, :]                               # [3, 2D]
            hproj = hc @ Wp.T + bp
            hn = hproj * (1.0 / np.sqrt((hproj ** 2).mean(-1, keepdims=True)
                                        + EPS))
            hxz = hn @ Win_eff                                  # [3, 2DI]
            xi_halo = np.ascontiguousarray(hxz[:, :DI].T)       # [DI, 3]
        in_maps1.append({
            "combT": cT.astype(ml_dtypes.bfloat16),
            "wp": Wp_augh, "win": Win_effh,
            "wcb": np.concatenate(
                [Wconv[:, 0, :], bconv[:, None], xi_halo], 1),
            "wx": Wxh, "wdt": Wdth,
        })

    res1 = _run_with_retry(nc1, in_maps1, _trace)
    if _timings is not None:
        _timings.append(res1)

    # ---- host reshard ----
    big = {}
    for b in range(B):
        cs = [b * 4 + q for q in range(4)]
        uT = np.concatenate([res1.results[c]["u_out"] for c in cs],
                            1).astype(np.float32)
        dtl = np.concatenate([res1.results[c]["delta_out"] for c in cs], 1)
        v = dtl + bdt[:, None]
        dT = (np.maximum(v, 0.0) + np.log1p(np.exp(-np.abs(v)))).astype(np.float32)
        zT = np.concatenate([res1.results[c]["z_out"] for c in cs], 1)
        bc = np.concatenate([res1.results[c]["bc_out"] for c in cs], 1)
        duT = dT * uT
        udT = (uT * Dparam[:, None]).astype(np.float32)
        # dbus[s, d, t] = du[d, t] * B[s, t]
        dbus = (duT[None, :, :] *
                bc[:DS, None, :].astype(np.float32)).astype(ml_dtypes.bfloat16)
        crep = np.ascontiguousarray(
            np.broadcast_to(bc[None, DS:, :].transpose(1, 0, 2), (DS, P, N)))
        big[b] = dict(dT=dT, zT=zT, dbus=dbus, udT=udT, crep=crep)

    in_maps2 = []
    for c in range(NCORES):
        b, j = divmod(c, 4)
        dsl = slice(j * DSH, (j + 1) * DSH)
        g = big[b]
        in_maps2.append({
            "delta": np.ascontiguousarray(g["dT"][dsl]),
            "ud": np.ascontiguousarray(g["udT"][dsl]),
            "z": np.ascontiguousarray(g["zT"][dsl]),
            "dbus": np.ascontiguousarray(g["dbus"][:, dsl]),
            "crep": g["crep"],
            "acol": np.ascontiguousarray(A[dsl]),
            "wout": np.ascontiguousarray(Wout[:, dsl].T).astype(
                ml_dtypes.bfloat16),
        })

    res2 = _run_with_retry(nc2, in_maps2, _trace)
    if _timings is not None:
        _timings.append(res2)

    # ---- host gather: sum d-shard partials, add skip ----
    out = np.empty((B, N, D), np.float32)
    for b in range(B):
        acc = res2.results[b * 4]["outp"].copy()
        for j in range(1, 4):
            acc += res2.results[b * 4 + j]["outp"]
        out[b] = acc.T + x[b]
    return out


# revision 25
# speedup vs baseline: 1.0271x; 1.0206x over previous
"""Trainium2 Bass kernel for nn_DeltaVisionMambaBlock.

Self-contained: takes FULL unsharded inputs, returns FULL output.

Decomposition across 8 NeuronCores, two SPMD launches, no collectives:
  Launch 1 (token-sharded: core = batch b x token-quarter q):
    diff-prologue proj (Wp) -> RMSNorm -> in_proj (Win) -> depthwise causal
    conv -> silu -> x_proj (Wx) -> dt_proj (Wdt) -> softplus.
    Channel-transposed layout throughout ([ch, tok]) so no PE transposes.
  Host: reshard [ch, tok] quarters into per-batch [DI, N]; du = delta*u;
    replicate B/C rows across partitions.
  Launch 2 (d_inner-sharded: core = batch b x d-quarter j):
    selective scan via DVE tensor_tensor_scan (time chunks of 512, state
    chained via last-column handoff), y = sum_s (h_s * C_s) accumulated in
    PSUM via identity matmuls, gating, out_proj (Wout) partials.
  Host: sum the 4 d-shard partials per batch, add skip connection.
"""
import sys

if "/opt/trn_rl_repo" not in sys.path:
    sys.path.insert(0, "/opt/trn_rl_repo")

import numpy as np
import ml_dtypes

import concourse.bass as bass
import concourse.tile as tile
from concourse import bacc, mybir
from concourse.bass_utils import run_bass_kernel_spmd
from concourse.masks import make_identity

F32 = mybir.dt.float32
BF16 = mybir.dt.bfloat16
AF = mybir.ActivationFunctionType
OP = mybir.AluOpType

B, N, D = 2, 2048, 768
DI, DS, DC, DTR = 1536, 16, 4, 48
EPS = 1e-5
NCORES = 8
TQ = N // 4            # 512 tokens per launch-1 core
DSH = DI // 4          # 384 d-inner channels per launch-2 core
KP = 13 * 128          # padded contraction dim (2D + bias row -> 1664)
P = 128
TCH = 512              # launch-2 time chunk
NTCH = N // TCH


# ----------------------------------------------------------------------
# Launch 1 program
# ----------------------------------------------------------------------
def _build_launch1():
    nc = bacc.Bacc("TRN2", target_bir_lowering=False, debug=False,
                   num_devices=NCORES)
    combT = nc.dram_tensor("combT", [KP, TQ], BF16, kind="ExternalInput").ap()
    wp = nc.dram_tensor("wp", [KP, D], BF16, kind="ExternalInput").ap()
    win = nc.dram_tensor("win", [D, 2 * DI], BF16, kind="ExternalInput").ap()
    wcb = nc.dram_tensor("wcb", [DI, DC + 1 + DC - 1], F32,
                         kind="ExternalInput").ap()
    wx = nc.dram_tensor("wx", [DI, DTR + 2 * DS], BF16, kind="ExternalInput").ap()
    wdt = nc.dram_tensor("wdt", [DTR, DI], BF16, kind="ExternalInput").ap()

    u_out = nc.dram_tensor("u_out", [DI, TQ], BF16, kind="ExternalOutput").ap()
    delta_out = nc.dram_tensor("delta_out", [DI, TQ], F32, kind="ExternalOutput").ap()
    z_out = nc.dram_tensor("z_out", [DI, TQ], BF16, kind="ExternalOutput").ap()
    bc_out = nc.dram_tensor("bc_out", [2 * DS, TQ], BF16, kind="ExternalOutput").ap()

    NK = KP // P          # 13
    ND = D // P           # 6
    NDI = DI // P         # 12

    with tile.TileContext(nc) as tc:
        with (
            tc.tile_pool(name="const", bufs=1) as cpool,
            tc.tile_pool(name="work", bufs=2) as work,
            tc.tile_pool(name="persist", bufs=1) as persist,
            tc.tile_pool(name="ps", bufs=4, space="PSUM") as pspool,
            tc.tile_pool(name="ps1", bufs=1, space="PSUM") as ps1pool,
        ):
            ones_k = cpool.tile([P, 1], F32, tag="ones_k")
            nc.vector.memset(ones_k[:], 1.0)
            ones_r = cpool.tile([1, P], F32, tag="ones_r")
            nc.vector.memset(ones_r[:], 1.0)

            normed_sb = []
            # ---- stage 1: projT = (Wp_aug.T)^T @ combT, m=channels ----
            with tc.tile_pool(name="s1", bufs=1) as s1:
                combT_sb = []
                wp_sb = []
                for k in range(NK):
                    t = s1.tile([P, TQ], BF16, tag=f"combT{k}")
                    nc.sync.dma_start(t[:], combT[k * P:(k + 1) * P, :])
                    combT_sb.append(t)
                    w = s1.tile([P, D], BF16, tag=f"wp{k}")
                    nc.scalar.dma_start(w[:], wp[k * P:(k + 1) * P, :])
                    wp_sb.append(w)

                proj_sb = []
                sq_sb = []
                for m in range(ND):
                    ps = pspool.tile([P, TQ], F32, tag="mm_ps")
                    for k in range(NK):
                        nc.tensor.matmul(ps[:], wp_sb[k][:, m * P:(m + 1) * P],
                                         combT_sb[k][:], start=(k == 0),
                                         stop=(k == NK - 1))
                    p_sb = s1.tile([P, TQ], F32, tag=f"proj{m}")
                    nc.vector.tensor_copy(p_sb[:], ps[:])
                    proj_sb.append(p_sb)
                    s_sb = s1.tile([P, TQ], F32, tag=f"sq{m}")
                    nc.scalar.activation(s_sb[:], ps[:], AF.Square)
                    sq_sb.append(s_sb)

                ssq_ps = ps1pool.tile([1, TQ], F32, tag="ssq")
                for m in range(ND):
                    nc.tensor.matmul(ssq_ps[:], ones_k[:], sq_sb[m][:],
                                     start=(m == 0), stop=(m == ND - 1))
                eps_t = cpool.tile([1, 1], F32, tag="eps")
                nc.vector.memset(eps_t[:], EPS)
                sqrt_sb = work.tile([1, TQ], F32, tag="sqrt")
                nc.scalar.activation(sqrt_sb[:], ssq_ps[:], AF.Sqrt,
                                     bias=eps_t[:, 0:1], scale=1.0 / D)
                rms_sb = work.tile([1, TQ], F32, tag="rms")
                nc.vector.reciprocal(rms_sb[:], sqrt_sb[:])
                rms_ps = ps1pool.tile([P, TQ], F32, tag="rmsb")
                nc.tensor.matmul(rms_ps[:], ones_r[:], rms_sb[:],
                                 start=True, stop=True)

                for m in range(ND):
                    nt = persist.tile([P, TQ], BF16, tag=f"normed{m}")
                    nc.vector.tensor_mul(nt[:], proj_sb[m][:], rms_ps[:])
                    normed_sb.append(nt)

            # ---- stage 2: xzT = (Win_eff.T)^T @ normedT ----
            xi_ext = []
            wcb_t = []
            for mi in range(NDI):
                wc = cpool.tile([P, DC + 1], F32, tag=f"wcb{mi}")
                nc.sync.dma_start(wc[:], wcb[mi * P:(mi + 1) * P, 0:DC + 1])
                wcb_t.append(wc)
                xe = persist.tile([P, (DC - 1) + TQ], F32, tag=f"xiext{mi}")
                nc.sync.dma_start(xe[:, 0:DC - 1],
                                  wcb[mi * P:(mi + 1) * P, DC + 1:])
                xi_ext.append(xe)

            with tc.tile_pool(name="s2", bufs=1) as s2:
                win_sb = []
                for k in range(ND):
                    w = s2.tile([P, 2 * DI], BF16, tag=f"win{k}")
                    nc.scalar.dma_start(w[:], win[k * P:(k + 1) * P, :])
                    win_sb.append(w)

                for mi in range(2 * NDI):
                    ps = pspool.tile([P, TQ], F32, tag="mm_ps")
                    for k in range(ND):
                        nc.tensor.matmul(ps[:],
                                         win_sb[k][:, mi * P:(mi + 1) * P],
                                         normed_sb[k][:], start=(k == 0),
                                         stop=(k == ND - 1))
                    if mi < NDI:
                        nc.scalar.activation(xi_ext[mi][:, DC - 1:], ps[:],
                                             AF.Copy)
                    else:
                        z_sb = work.tile([P, TQ], BF16, tag="z")
                        nc.scalar.activation(z_sb[:], ps[:], AF.Copy)
                        j = mi - NDI
                        nc.gpsimd.dma_start(z_out[j * P:(j + 1) * P, :], z_sb[:])

            # ---- conv + silu -> u ----
            u_sb = []
            for mi in range(NDI):
                acc = work.tile([P, TQ], F32, tag="cacc")
                nc.vector.tensor_scalar(acc[:], xi_ext[mi][:, 0:TQ],
                                        wcb_t[mi][:, 0:1], None, OP.mult)
                for j in range(1, DC):
                    acc2 = work.tile([P, TQ], F32, tag="cacc")
                    nc.vector.scalar_tensor_tensor(
                        acc2[:], xi_ext[mi][:, j:j + TQ], wcb_t[mi][:, j:j + 1],
                        acc[:], OP.mult, OP.add)
                    acc = acc2
                ut = persist.tile([P, TQ], BF16, tag=f"u{mi}")
                nc.scalar.activation(ut[:], acc[:], AF.Silu,
                                     bias=wcb_t[mi][:, DC:DC + 1])
                u_sb.append(ut)
                nc.gpsimd.dma_start(u_out[mi * P:(mi + 1) * P, :], ut[:])

            # ---- x_dbl = (Wx.T)^T @ uT ----
            wx_sb = []
            for k in range(NDI):
                w = cpool.tile([P, DTR + 2 * DS], BF16, tag=f"wx{k}")
                nc.sync.dma_start(w[:], wx[k * P:(k + 1) * P, :])
                wx_sb.append(w)
            xdbl_ps = ps1pool.tile([DTR + 2 * DS, TQ], F32, tag="xdbl")
            for k in range(NDI):
                nc.tensor.matmul(xdbl_ps[:], wx_sb[k][:], u_sb[k][:],
                                 start=(k == 0), stop=(k == NDI - 1))
            xdbl_sb = persist.tile([DTR + 2 * DS, TQ], BF16, tag="xdbl_sb")
            nc.scalar.activation(xdbl_sb[:], xdbl_ps[:], AF.Copy)
            nc.sync.dma_start(bc_out[:], xdbl_sb[DTR:, :])

            # ---- dt_lin = Wdt @ dt (softplus+bias on host) ----
            wdt_sb = cpool.tile([DTR, DI], BF16, tag="wdt")
            nc.sync.dma_start(wdt_sb[:], wdt[:])
            for mi in range(NDI):
                ps = pspool.tile([P, TQ], F32, tag="mm_ps")
                nc.tensor.matmul(ps[:], wdt_sb[:, mi * P:(mi + 1) * P],
                                 xdbl_sb[0:DTR, :], start=True, stop=True)
                d_sb = work.tile([P, TQ], F32, tag="delta")
                nc.scalar.activation(d_sb[:], ps[:], AF.Copy)
                nc.gpsimd.dma_start(delta_out[mi * P:(mi + 1) * P, :], d_sb[:])

    nc.compile()
    return nc


# ----------------------------------------------------------------------
# Launch 2 program
# ----------------------------------------------------------------------
def _build_launch2():
    nc = bacc.Bacc("TRN2", target_bir_lowering=False, debug=False,
                   num_devices=NCORES)
    delta = nc.dram_tensor("delta", [DSH, N], F32, kind="ExternalInput").ap()
    ud = nc.dram_tensor("ud", [DSH, N], F32, kind="ExternalInput").ap()
    z = nc.dram_tensor("z", [DSH, N], BF16, kind="ExternalInput").ap()
    dbus = nc.dram_tensor("dbus", [DS, DSH, N], BF16, kind="ExternalInput").ap()
    crep = nc.dram_tensor("crep", [DS, P, N], BF16, kind="ExternalInput").ap()
    acol = nc.dram_tensor("acol", [DSH, DS], F32, kind="ExternalInput").ap()
    wout = nc.dram_tensor("wout", [DSH, D], BF16, kind="ExternalInput").ap()
    outp = nc.dram_tensor("outp", [D, N], F32, kind="ExternalOutput").ap()

    NCT = DSH // P        # 3

    with tile.TileContext(nc) as tc:
        with (
            tc.tile_pool(name="const", bufs=1) as cpool,
            tc.tile_pool(name="inp", bufs=1) as inp,
            tc.tile_pool(name="bc", bufs=2) as bcpool,
            tc.tile_pool(name="work", bufs=2) as work,
            tc.tile_pool(name="yz", bufs=1) as yzpool,
            tc.tile_pool(name="ps", bufs=2, space="PSUM") as pspool,
        ):
            ident = cpool.tile([P, P], BF16, tag="ident")
            make_identity(nc, ident[:])

            delta_t, ud_t, z_t, acol_t, wout_t = [], [], [], [], []
            for ct in range(NCT):
                r = slice(ct * P, (ct + 1) * P)
                t = inp.tile([P, N], F32, tag=f"dlt{ct}")
                nc.sync.dma_start(t[:], delta[r, :])
                delta_t.append(t)
                t = inp.tile([P, N], F32, tag=f"ud{ct}")
                nc.sync.dma_start(t[:], ud[r, :])
                ud_t.append(t)
                t = inp.tile([P, N], BF16, tag=f"z{ct}")
                nc.sync.dma_start(t[:], z[r, :])
                z_t.append(t)
                t = inp.tile([P, DS], F32, tag=f"acol{ct}")
                nc.sync.dma_start(t[:], acol[r, :])
                acol_t.append(t)
                t = inp.tile([P, D], BF16, tag=f"wout{ct}")
                nc.sync.dma_start(t[:], wout[r, :])
                wout_t.append(t)
            yz_t = [yzpool.tile([P, N], BF16, tag=f"yz{ct}", name=f"yz{ct}")
                    for ct in range(NCT)]

            # ct passes: {0,1} then {2} -- two 2048-wide psum accumulators max
            for cts in ([0, 1], [2]):
                y_ps = {}
                for ct in cts:
                    y_ps[ct] = pspool.tile([P, N], F32, tag="big",
                                           name=f"yps{ct}")
                for s in range(DS):
                    c_sb = bcpool.tile([P, N], BF16, tag="c", bufs=3)
                    nc.scalar.dma_start(c_sb[:], crep[s, :, :])
                    for ct in cts:
                        dA = work.tile([P, N], F32, tag="dA", bufs=3)
                        nc.scalar.activation(dA[:], delta_t[ct][:], AF.Exp,
                                             scale=acol_t[ct][:, s:s + 1])
                        dBu = work.tile([P, N], BF16, tag="dBu", bufs=3)
                        nc.gpsimd.dma_start(
                            dBu[:], dbus[s, ct * P:(ct + 1) * P, :])
                        h = work.tile([P, N], BF16, tag="h")
                        nc.vector.tensor_tensor_scan(
                            h[:], dA[:], dBu[:], 0.0, OP.mult, OP.add)
                        hc = work.tile([P, N], BF16, tag="hc", bufs=3)
                        nc.vector.tensor_mul(hc[:], h[:], c_sb[:])
                        for nch in range(N // 512):
                            ns = slice(nch * 512, (nch + 1) * 512)
                            nc.tensor.matmul(y_ps[ct][:, ns], ident[:],
                                             hc[:, ns], start=(s == 0),
                                             stop=(s == DS - 1))
                for ct in cts:
                    yD = work.tile([P, N], F32, tag="yD", bufs=1)
                    nc.vector.tensor_add(yD[:], ud_t[ct][:], y_ps[ct][:])
                    silz = work.tile([P, N], F32, tag="silz", bufs=1)
                    nc.scalar.activation(silz[:], z_t[ct][:], AF.Silu)
                    nc.vector.tensor_mul(yz_t[ct][:], yD[:], silz[:])

            # ---- out_proj partials ----
            for mj in range(D // P):
                o_sb = work.tile([P, N], F32, tag="o_sb", bufs=2)
                for nch in range(NTCH):
                    ns = slice(nch * TCH, (nch + 1) * TCH)
                    ps = pspool.tile([P, TCH], F32, tag="big", name="o_ps")
                    for ct in range(NCT):
                        nc.tensor.matmul(ps[:],
                                         wout_t[ct][:, mj * P:(mj + 1) * P],
                                         yz_t[ct][:, ns], start=(ct == 0),
                                         stop=(ct == NCT - 1),
                                         skip_group_check=True)
                    nc.scalar.activation(o_sb[:, ns], ps[:], AF.Copy)
                nc.gpsimd.dma_start(outp[mj * P:(mj + 1) * P, :], o_sb[:])

    nc.compile()
    return nc


_CACHE = {}


def _run_with_retry(nc, in_maps, trace, attempts=3):
    last = None
    for i in range(attempts):
        try:
            return run_bass_kernel_spmd(nc, in_maps,
                                        core_ids=list(range(NCORES)),
                                        trace=trace)
        except Exception as e:  # transient NRT_EXEC_UNIT_UNRECOVERABLE
            last = e
            import time
            time.sleep(2.0 * (i + 1))
    raise last


def _get_programs():
    if "nc1" not in _CACHE:
        _CACHE["nc1"] = _build_launch1()
        _CACHE["nc2"] = _build_launch2()
    return _CACHE["nc1"], _CACHE["nc2"]


# ----------------------------------------------------------------------
# Host orchestration
# ----------------------------------------------------------------------
def kernel(x, Wp, bp, gamma, Win, Wconv, bconv, Wx, Wdt, bdt, A_log,
           Dparam, Wout, _timings=None, _trace=False):
    x = np.asarray(x, np.float32)
    f = lambda a: np.ascontiguousarray(np.asarray(a, np.float32))
    Wp, bp, gamma = f(Wp), f(bp), f(gamma)
    Win, Wconv, bconv = f(Win), f(Wconv), f(bconv)
    Wx, Wdt, bdt = f(Wx), f(Wdt), f(bdt)
    A_log, Dparam, Wout = f(A_log), f(Dparam), f(Wout)

    nc1, nc2 = _get_programs()

    # ---- host prep for launch 1 ----
    prev = np.concatenate([np.zeros((B, 1, D), np.float32), x[:, :-1, :]], 1)
    comb = np.concatenate([x, x - prev], -1)                    # [B,N,2D]
    Wp_aug = np.zeros((KP, D), np.float32)
    Wp_aug[:2 * D, :] = Wp.T
    Wp_aug[2 * D, :] = bp
    Win_eff = (Win * gamma[None, :]).T.copy()                   # [768, 3072]
    Wp_augh = Wp_aug.astype(ml_dtypes.bfloat16)
    Win_effh = Win_eff.astype(ml_dtypes.bfloat16)
    Wxh = Wx.T.copy().astype(ml_dtypes.bfloat16)
    Wdth = Wdt.T.copy().astype(ml_dtypes.bfloat16)
    A = -np.exp(A_log)                                          # [DI, DS]

    # host-computed halo xi (3 tokens before each quarter boundary)
    halo_idx = [TQ - (DC - 1) + j for j in range(DC - 1)]
    in_maps1 = []
    for c in range(NCORES):
        b, q = divmod(c, 4)
        tok = slice(q * TQ, (q + 1) * TQ)
        cT = np.zeros((KP, TQ), np.float32)
        cT[:2 * D, :] = comb[b, tok, :].T
        cT[2 * D, :] = 1.0
        if q == 0:
            xi_halo = np.zeros((DI, DC - 1), np.float32)
        else:
            htok = [q * TQ - (DC - 1) + j for j in range(DC - 1)]
            hc = comb[b, htok, :]                               # [3, 2D]
            hproj = hc @ Wp.T + bp
            hn = hproj * (1.0 / np.sqrt((hproj ** 2).mean(-1, keepdims=True)
                                        + EPS))
            hxz = hn @ Win_eff                                  # [3, 2DI]
            xi_halo = np.ascontiguousarray(hxz[:, :DI].T)       # [DI, 3]
        in_maps1.append({
            "combT": cT.astype(ml_dtypes.bfloat16),
            "wp": Wp_augh, "win": Win_effh,
            "wcb": np.concatenate(
                [Wconv[:, 0, :], bconv[:, None], xi_halo], 1),
            "wx": Wxh, "wdt": Wdth,
        })

    res1 = _run_with_retry(nc1, in_maps1, _trace)
    if _timings is not None:
        _timings.append(res1)

    # ---- host reshard ----
    big = {}
    for b in range(B):
        cs = [b * 4 + q for q in range(4)]
        uT = np.concatenate([res1.results[c]["u_out"] for c in cs],
                            1).astype(np.float32)
        dtl = np.concatenate([res1.results[c]["delta_out"] for c in cs], 1)
        v = dtl + bdt[:, None]
        dT = (np.maximum(v, 0.0) + np.log1p(np.exp(-np.abs(v)))).astype(np.float32)
        zT = np.concatenate([res1.results[c]["z_out"] for c in cs], 1)
        bc = np.concatenate([res1.results[c]["bc_out"] for c in cs], 1)
        duT = dT * uT
        udT = (uT * Dparam[:, None]).astype(np.float32)
        # dbus[s, d, t] = du[d, t] * B[s, t]
        dbus = (duT[None, :, :] *
                bc[:DS, None, :].astype(np.float32)).astype(ml_dtypes.bfloat16)
        crep = np.ascontiguousarray(
            np.broadcast_to(bc[None, DS:, :].transpose(1, 0, 2), (DS, P, N)))
        big[b] = dict(dT=dT, zT=zT, dbus=dbus, udT=udT, crep=crep)

    in_maps2 = []
    for c in range(NCORES):
        b, j = divmod(c, 4)
        dsl = slice(j * DSH, (j + 1) * DSH)
        g = big[b]
        in_maps2.append({
            "delta": np.ascontiguousarray(g["dT"][dsl]),
            "ud": np.ascontiguousarray(g["udT"][dsl]),
            "z": np.ascontiguousarray(g["zT"][dsl]),
            "dbus": np.ascontiguousarray(g["dbus"][:, dsl]),
            "crep": g["crep"],
            "acol": np.ascontiguousarray(A[dsl]),
            "wout": np.ascontiguousarray(Wout[:, dsl].T).astype(
                ml_dtypes.bfloat16),
        })

    res2 = _run_with_retry(nc2, in_maps2, _trace)
    if _timings is not None:
        _timings.append(res2)

    # ---- host gather: sum d-shard partials, add skip ----
    out = np.empty((B, N, D), np.float32)
    for b in range(B):
        acc = res2.results[b * 4]["outp"].copy()
        for j in range(1, 4):
            acc += res2.results[b * 4 + j]["outp"]
        out[b] = acc.T + x[b]
    return out
